# revision 1
# baseline (speedup 1.0000x reference)
"""Multi-head attention (B=8, H=8, S=1024, d=128) on 8 TRN2 NeuronCores.

Strategy
--------
- Data-parallel over batch: core i handles batch i (8 cores, B=8).
- Host-side prep (layout only): per batch, compact keys/values to the
  seq_mask-selected rows (zero-padded to a multiple of 128 -> kt_tiles
  k-tiles), pre-transpose Q and compacted K so the contraction dim (d)
  lands on SBUF partitions, and cast matmul operands to fp16 (10-bit
  mantissa, same precision class as the PE's TF32 path, but 2-byte
  weight loads that background-prefetch). An indicator matrix
  ind[k, 32] (1 for real keys) rides along for the softmax denominator.
- Device math per (head h, k-tile kt), all matmuls as column-tiled M=64
  (or M=32) pairs on disjoint PE column groups + disjoint PSUM banks so
  the two streams run concurrently:
    logitsT[k, q]  = K^T[:, kt].T @ Q^T           (PE, contraction d)
    W^T[k, q]      = exp(logitsT * d^-0.5)        (ACT, PSUM -> SBUF fp16)
    outT[d, q]    += V[kt].T   @ W^T              (PE, accum over kt)
    den[q]        += ind[kt].T @ W^T              (PE, accum over kt)
  then copy outT/den to SBUF (DVE) and DMA out; the division happens on
  the host (DVE reciprocal is microcoded and slow).
  The learned scalar bias b cancels in softmax (shift invariance) and
  the -1e30 masking is equivalent to dropping masked keys (exp -> 0),
  which the compaction does exactly.
- Host-side unshard: out[b] = outT.T / den (plus uniform-average
  fallback for a fully-masked batch, where reference degenerates to a
  uniform softmax).
"""
from contextlib import ExitStack

import numpy as np

import concourse.bacc as bacc
import concourse.mybir as mybir
import concourse.tile as tile
from concourse.bass_utils import run_bass_kernel_spmd

F32 = mybir.dt.float32
F32R = mybir.dt.float32r
F16 = mybir.dt.float16

B, S, D, H = 8, 1024, 1024, 8
DH = D // H              # 128, head dim = one partition tile
SCALE = float(DH) ** -0.5
NQC = S // 512           # q chunks of 512 for AV/normalize

_NC_CACHE: dict[tuple, object] = {}

# build options (overridable for profiling experiments)
OPTS: dict = {}


def _build(kt_tiles: int, opts: dict | None = None):
    """Build + compile the per-core kernel for `kt_tiles` 128-wide key tiles."""
    opts = opts or {}
    pl_bufs = opts.get("pl_bufs", 2)
    po_bufs = opts.get("po_bufs", 1)
    copy_eng = opts.get("copy_eng", "vector")   # engine for PSUM->SBUF copies
    store_eng = opts.get("store_eng", "sync")  # engine issuing output DMAs
    KP = kt_tiles * 128
    nc = bacc.Bacc("TRN2", target_bir_lowering=False, debug=False)

    q_t = nc.dram_tensor("q_t", [D, S], F16, kind="ExternalInput")
    k_t = nc.dram_tensor("k_t", [D, KP], F16, kind="ExternalInput")
    v_c = nc.dram_tensor("v_c", [KP, D], F16, kind="ExternalInput")
    ind = nc.dram_tensor("ind", [KP, 32], F16, kind="ExternalInput")
    out_t = nc.dram_tensor("out_t", [D, S], F32, kind="ExternalOutput")
    den_t = nc.dram_tensor("den_t", [H, 33, S], F32, kind="ExternalOutput")

    with tile.TileContext(nc) as tc, ExitStack() as ctx:
        sb_in = ctx.enter_context(tc.tile_pool(name="sb_in", bufs=3))
        sb_ind = ctx.enter_context(tc.tile_pool(name="sb_ind", bufs=1))
        sb_w = ctx.enter_context(tc.tile_pool(name="sb_w", bufs=6))
        sb_out = ctx.enter_context(tc.tile_pool(name="sb_out", bufs=4))
        ps_l = ctx.enter_context(
            tc.tile_pool(name="ps_l", bufs=pl_bufs, space="PSUM"))
        ps_o = ctx.enter_context(
            tc.tile_pool(name="ps_o", bufs=po_bufs, space="PSUM"))
        ps_d = ctx.enter_context(tc.tile_pool(name="ps_d", bufs=1, space="PSUM"))

        store = {"gpsimd": nc.gpsimd, "scalar": nc.scalar, "sync": nc.sync}[store_eng]

        def copy_op(dst, src):
            if copy_eng == "vector":
                nc.vector.tensor_copy(dst, src)
            else:
                nc.scalar.copy(dst, src)

        ind_sb = None

        for h in range(H):
            hs = h * DH
            # first chunks first: the kernel's first matmul needs
            # kth[:, :128] and qth[:, :512] only
            kth = sb_in.tile([128, KP], F16, tag="kth")
            nc.sync.dma_start(kth[:, 0:128], k_t.ap()[hs:hs + DH, 0:128])
            qth = sb_in.tile([128, S], F16, tag="qth")
            nc.sync.dma_start(qth[:, 0:512], q_t.ap()[hs:hs + DH, 0:512])
            if KP > 128:
                nc.sync.dma_start(kth[:, 128:], k_t.ap()[hs:hs + DH, 128:])
            nc.sync.dma_start(qth[:, 512:], q_t.ap()[hs:hs + DH, 512:])
            if ind_sb is None:
                # indicator tiles: [128(k), 32] per k-tile, concatenated on
                # the free dim. 32 columns keeps their LDWEIGHTS cheap.
                ind_sb = sb_ind.tile([128, kt_tiles * 32], F16)
                nc.sync.dma_start(
                    ind_sb[:].rearrange("p (t c) -> p t c", c=32),
                    ind.ap().rearrange("(t p) c -> p t c", p=128),
                )
            # V for this head: [128(k), 128(d)] tiles concatenated on free dim
            vh = sb_in.tile([128, KP], F16, tag="vh")
            nc.sync.dma_start(
                vh[:].rearrange("p (t c) -> p t c", c=DH),
                v_c.ap()[:, hs:hs + DH].rearrange("(t p) c -> p t c", p=128),
            )

            po = ps_o.tile([128, S], F32, tag="po")    # outT accum [d, q]
            # denominator: [0:32, 0:512] = q-chunk 0, [32:64, 512:] = q-chunk 1
            pd = ps_d.tile([64, S], F32, tag="pd")

            # Column-tiled matmul pairs: two M=64 matmuls on disjoint PE
            # column groups AND disjoint PSUM banks stream concurrently.
            # Pair diagonally across (column-half, q-chunk) so banks differ.
            s0, s1 = slice(0, 512), slice(512, 1024)
            wts = []

            def emit_qk(kt):
                pl = ps_l.tile([128, S], F32, tag="pl", name=f"pl_{h}_{kt}")
                ks = kt * 128
                kA, kB = slice(ks, ks + 64), slice(ks + 64, ks + 128)
                # pair 1: (half A, qc0/bank0) + (half B, qc1/bank1)
                nc.tensor.matmul(pl[0:64, s0], kth[:, kA], qth[:, s0])
                nc.tensor.matmul(pl[64:128, s1], kth[:, kB], qth[:, s1])
                # pair 2: (half B, qc0/bank0) + (half A, qc1/bank1)
                nc.tensor.matmul(pl[64:128, s0], kth[:, kB], qth[:, s0])
                nc.tensor.matmul(pl[0:64, s1], kth[:, kA], qth[:, s1])
                wt = sb_w.tile([128, S], F16, tag="wt", name=f"wt_{h}_{kt}")
                nc.scalar.activation(
                    wt[:], pl[:], mybir.ActivationFunctionType.Exp, scale=SCALE
                )
                wts.append(wt)

            emit_qk(0)
            for kt in range(kt_tiles):
                if kt + 1 < kt_tiles:
                    emit_qk(kt + 1)
                wt = wts[kt]
                ks = kt * 128
                dA, dB = slice(ks, ks + 64), slice(ks + 64, ks + 128)
                first, last = kt == 0, kt == kt_tiles - 1
                ic = slice(kt * 32, kt * 32 + 32)
                order = opts.get("mm_order", "dAA")
                mm_den = [
                    (pd[0:32, s0], ind_sb[:, ic], wt[:, s0]),
                    (pd[32:64, s1], ind_sb[:, ic], wt[:, s1]),
                ]
                mm_av1 = [
                    (po[0:64, s0], vh[:, dA], wt[:, s0]),
                    (po[64:128, s1], vh[:, dB], wt[:, s1]),
                ]
                mm_av2 = [
                    (po[64:128, s0], vh[:, dB], wt[:, s0]),
                    (po[0:64, s1], vh[:, dA], wt[:, s1]),
                ]
                seqs = {"dAA": mm_den + mm_av1 + mm_av2,
                        "AdA": mm_av1 + mm_den + mm_av2,
                        "AAd": mm_av1 + mm_av2 + mm_den}[order]
                for out_ap, w_ap, r_ap in seqs:
                    nc.tensor.matmul(out_ap, w_ap, r_ap, start=first, stop=last)

            # denominator first (releases pd for the next head's den matmuls);
            # rows 0 and 32 carry the real values, host picks them out
            dsb = sb_out.tile([33, S], F32, tag="dsb")
            nc.vector.tensor_copy(dsb[:], pd[0:33, :])
            store.dma_start(den_t.ap()[h, :, :], dsb[:])
            # numerator to SBUF, divide on host
            osb = sb_out.tile([128, S], F32, tag="osb")
            copy_op(osb[:], po[:])
            store.dma_start(out_t.ap()[hs:hs + DH, :], osb[:])

    nc.compile()
    return nc


def kernel(memory, query, seq_mask, b):
    memory = np.ascontiguousarray(memory, dtype=np.float32)
    query = np.ascontiguousarray(query, dtype=np.float32)
    seq_mask = np.asarray(seq_mask)
    assert memory.shape == (B, S, 2 * D) and query.shape == (B, S, D)

    counts = [int(np.count_nonzero(seq_mask[i])) for i in range(B)]
    kp = max(max(counts), 1)
    kp = ((kp + 127) // 128) * 128
    kt_tiles = kp // 128

    key = (kt_tiles, tuple(sorted(OPTS.items())))
    if key not in _NC_CACHE:
        _NC_CACHE[key] = _build(kt_tiles, OPTS)
    nc = _NC_CACHE[key]

    q_t = np.ascontiguousarray(query.transpose(0, 2, 1)).astype(np.float16)
    in_maps = []
    for i in range(B):
        idx = np.flatnonzero(seq_mask[i])
        nb = len(idx)
        ktb = np.zeros((D, kp), dtype=np.float16)
        vcb = np.zeros((kp, D), dtype=np.float16)
        indb = np.zeros((kp, 32), dtype=np.float16)
        if nb:
            ktb[:, :nb] = memory[i, idx, :D].T
            vcb[:nb] = memory[i, idx, D:]
            indb[:nb] = 1.0
        in_maps.append({"q_t": q_t[i], "k_t": ktb, "v_c": vcb, "ind": indb})

    res = run_bass_kernel_spmd(nc, in_maps, list(range(B)))
    out = np.empty((B, S, D), dtype=np.float32)
    for i in range(B):
        num = res.results[i]["out_t"]            # [D, S] = [(h d), q]
        dd = res.results[i]["den_t"]             # [H, 33, S]
        den = np.concatenate([dd[:, 0, 0:512], dd[:, 32, 512:1024]], axis=1)
        with np.errstate(divide="ignore", invalid="ignore"):
            out[i] = (num.reshape(H, DH, S) / den[:, None, :]).reshape(D, S).T
        if counts[i] == 0:
            # all keys masked: reference softmax degenerates to uniform
            out[i] = memory[i, :, D:].mean(axis=0)[None, :]
    return out



# revision 6
# speedup vs baseline: 1.2498x; 1.2498x over previous
"""Multi-head attention (B=8, H=8, S=1024, d=128) on 8 TRN2 NeuronCores.

Strategy
--------
- Data-parallel over batch: core i handles batch i (8 cores, B=8).
- Host-side prep (layout only): per batch, compact keys/values to the
  seq_mask-selected rows (zero-padded to a multiple of 128 -> kt_tiles
  k-tiles), pre-transpose Q and compacted K so the contraction dim (d)
  lands on SBUF partitions, and cast matmul operands to fp16. V is
  augmented per head with a 129th "indicator" column (1 for real keys,
  0 for padding) so the softmax denominator falls out of the AV matmul.
- Device math per head h:
    logitsT[k, q] = K_h^T.T @ Q_h^T          (PE, M=128 k-tiles, N=512)
    W^T[k, q]     = exp(logitsT * d^-0.5)    (ACT, PSUM -> SBUF fp16)
    out[q, 129]   = sum_kt W^T[kt,qtile].T @ [V_h[kt] | ind[kt]]
                                             (PE, M=128 q-tiles, N=129,
                                              PSUM accumulation over kt;
                                              col 128 = denominator)
    osb[q, d]     = out[:, :128] * recip(out[:, 128])  (DVE)
  The learned scalar bias b cancels in softmax (shift invariance) and
  the -1e30 masking is equivalent to dropping masked keys, which the
  compaction does exactly.
- Output per head is DMA'd as a contiguous [128, 1024] fp16 block
  ([q-within-tile, (q-tile, d)]); the host reassembles [S, D] and
  handles the degenerate all-masked batch (uniform average).
"""
from contextlib import ExitStack

import numpy as np

import concourse.bacc as bacc
import concourse.mybir as mybir
import concourse.tile as tile
from concourse.bass_utils import run_bass_kernel_spmd

F32 = mybir.dt.float32
F16 = mybir.dt.float16

B, S, D, H = 8, 1024, 1024, 8
DH = D // H              # 128, head dim = one partition tile
SCALE = float(DH) ** -0.5
NQT = S // 128           # 8 q-tiles per head

_NC_CACHE: dict[tuple, object] = {}

# build options (overridable for profiling experiments)
OPTS: dict = {}


def _build(kt_tiles: int, opts: dict | None = None):
    """Build + compile the per-core kernel for `kt_tiles` 128-wide key tiles."""
    opts = opts or {}
    pl_bufs = opts.get("pl_bufs", 2)
    w_bufs = opts.get("w_bufs", 8)
    o_bufs = opts.get("o_bufs", 2)
    KP = kt_tiles * 128
    VW = H * 129             # per-k-row width of augmented V
    nc = bacc.Bacc("TRN2", target_bir_lowering=False, debug=False)

    q_t = nc.dram_tensor("q_t", [D, S], F16, kind="ExternalInput")
    k_t = nc.dram_tensor("k_t", [D, KP], F16, kind="ExternalInput")
    v_a = nc.dram_tensor("v_a", [KP, VW], F16, kind="ExternalInput")
    out_t = nc.dram_tensor("out_t", [H, 128, S], F16, kind="ExternalOutput")

    # ps_out layout: 3 bank-aligned groups of q-tiles (3+3+2), each q-tile
    # owning 129 columns (128 dims + denominator). Offsets within the
    # [128, 1536] tile; a 129-wide matmul output may not cross a PSUM bank.
    def po_off(qi):
        g, j = divmod(qi, 3)
        return g * 512 + j * 129

    with tile.TileContext(nc) as tc, ExitStack() as ctx:
        sb_k = ctx.enter_context(tc.tile_pool(name="sb_k", bufs=1))
        sb_q = ctx.enter_context(tc.tile_pool(name="sb_q", bufs=1))
        sb_v = ctx.enter_context(tc.tile_pool(name="sb_v", bufs=1))
        sb_w = ctx.enter_context(tc.tile_pool(name="sb_w", bufs=w_bufs))
        sb_o = ctx.enter_context(tc.tile_pool(name="sb_o", bufs=o_bufs))
        ps_l = ctx.enter_context(
            tc.tile_pool(name="ps_l", bufs=pl_bufs, space="PSUM"))
        ps_o = ctx.enter_context(tc.tile_pool(name="ps_o", bufs=1, space="PSUM"))

        kall = sb_k.tile([128, H * KP], F16)
        qall = sb_q.tile([128, H * S], F16)
        vall = sb_v.tile([128, kt_tiles * VW], F16)

        # Input DMAs, split per head / per k-tile and interleaved so the
        # tiles head 0 needs first are issued first on the sync queue.
        def dma_k(h):
            nc.sync.dma_start(
                kall[:, h * KP:(h + 1) * KP], k_t.ap()[h * DH:(h + 1) * DH, :])

        def dma_q(h):
            nc.sync.dma_start(
                qall[:, h * S:(h + 1) * S], q_t.ap()[h * DH:(h + 1) * DH, :])

        def dma_v(kt):
            # straight 2D copy, 2064B rows
            nc.sync.dma_start(
                vall[:, kt * VW:(kt + 1) * VW],
                v_a.ap()[kt * 128:(kt + 1) * 128, :])

        dma_k(0); dma_q(0); dma_v(0)
        dma_k(1); dma_q(1); dma_v(1)
        dma_k(2); dma_q(2)
        for kt in range(2, kt_tiles):
            dma_v(kt)
        for h in range(3, H):
            dma_k(h); dma_q(h)

        for h in range(H):
            hq = h * S
            wts = []
            for kt in range(kt_tiles):
                pl = ps_l.tile([128, S], F32, tag="pl", name=f"pl_{h}_{kt}")
                lhsT = kall[:, h * KP + kt * 128: h * KP + (kt + 1) * 128]
                nc.tensor.matmul(pl[:, 0:512], lhsT, qall[:, hq:hq + 512],
                                 start=True, stop=True)
                nc.tensor.matmul(pl[:, 512:1024], lhsT,
                                 qall[:, hq + 512:hq + 1024],
                                 start=True, stop=True)
                wt = sb_w.tile([128, S], F16, tag="wt", name=f"wt_{h}_{kt}")
                nc.scalar.activation(
                    wt[:], pl[:], mybir.ActivationFunctionType.Exp, scale=SCALE)
                wts.append(wt)

            po = ps_o.tile([128, 1536], F32, tag="po", name=f"po_{h}")
            for kt in range(kt_tiles):
                first, last = kt == 0, kt == kt_tiles - 1
                rhs = vall[:, kt * VW + h * 129: kt * VW + (h + 1) * 129]
                for qi in range(NQT):
                    off = po_off(qi)
                    # start=True clears the has_written bits of the WHOLE
                    # bank, so only the first matmul touching each bank may
                    # carry it; the other regions' first writes rely on
                    # their (now cleared) bits selecting overwrite mode.
                    nc.tensor.matmul(
                        po[:, off:off + 129],
                        wts[kt][:, qi * 128:(qi + 1) * 128],
                        rhs, start=first and qi % 3 == 0, stop=last,
                        skip_group_check=True)

            # Epilogue: single PSUM->SBUF copy (releases po), then
            # normalize: osb[:, qi] = num_qi * recip(den_qi).
            oall = sb_o.tile([128, 1536], F32, tag="oall", name=f"oall_{h}")
            nc.vector.tensor_copy(oall[:], po[:])
            den9 = oall.rearrange("p (g x) -> p g x", g=3)[:, :, 128:512:129]
            rst = sb_o.tile([128, 9], F32, tag="rst", name=f"rst_{h}")
            nc.vector.reciprocal(rst[:], den9)
            osb = sb_o.tile([128, S], F16, tag="osb", name=f"osb_{h}")
            for qi in range(NQT):
                g, j = divmod(qi, 3)
                off = po_off(qi)
                nc.vector.tensor_scalar_mul(
                    osb[:, qi * 128:(qi + 1) * 128],
                    oall[:, off:off + 128], rst[:, g * 3 + j:g * 3 + j + 1])
            nc.gpsimd.dma_start(out_t.ap()[h], osb[:])

    nc.compile()
    return nc


def kernel(memory, query, seq_mask, b):
    memory = np.ascontiguousarray(memory, dtype=np.float32)
    query = np.ascontiguousarray(query, dtype=np.float32)
    seq_mask = np.asarray(seq_mask)
    assert memory.shape == (B, S, 2 * D) and query.shape == (B, S, D)

    counts = [int(np.count_nonzero(seq_mask[i])) for i in range(B)]
    kp = max(max(counts), 1)
    kp = ((kp + 127) // 128) * 128
    kt_tiles = kp // 128

    key = (kt_tiles, tuple(sorted(OPTS.items())))
    if key not in _NC_CACHE:
        _NC_CACHE[key] = _build(kt_tiles, OPTS)
    nc = _NC_CACHE[key]

    q_t = np.ascontiguousarray(query.transpose(0, 2, 1)).astype(np.float16)
    in_maps = []
    for i in range(B):
        idx = np.flatnonzero(seq_mask[i])
        nb = len(idx)
        ktb = np.zeros((D, kp), dtype=np.float16)
        vab = np.zeros((kp, H, 129), dtype=np.float16)
        if nb:
            ktb[:, :nb] = memory[i, idx, :D].T
            vab[:nb, :, :128] = memory[i, idx, D:].reshape(nb, H, DH)
            vab[:nb, :, 128] = 1.0
        in_maps.append(
            {"q_t": q_t[i], "k_t": ktb, "v_a": vab.reshape(kp, H * 129)})

    res = run_bass_kernel_spmd(nc, in_maps, list(range(B)))
    out = np.empty((B, S, D), dtype=np.float32)
    for i in range(B):
        o = res.results[i]["out_t"].astype(np.float32)   # [H, 128, S]
        # [h, p, (qi d)] -> [qi, p, h, d] -> [S, D]
        out[i] = o.reshape(H, 128, NQT, DH).transpose(2, 1, 0, 3).reshape(S, D)
        if counts[i] == 0:
            # all keys masked: reference softmax degenerates to uniform
            out[i] = memory[i, :, D:].mean(axis=0)[None, :]
    return out
